# revision 25
# baseline (speedup 1.0000x reference)
"""Trainium2 Bass kernel for 16->16 channel 3x3 VALID conv on [16,1536,1536].

out[co, y, x] = sum_{ci,dy,dx} W[co,ci,dy,dx] * X[ci, y+dy, x+dx] + sum(bias)

Strategy (8-core data parallel over H, halo of 2 rows):
  Each core computes 192 output rows from a 194-row input shard, in 32 blocks
  of R=6 output rows. Per block, an SBUF "window" of 8 input rows x 16
  channels is laid out as [128, 1536] with partition p = k*16 + ci (k = row
  in window, k-major). The conv becomes 3 accumulating matmuls (one per
  kernel column dx) whose dx shift is a free-dim offset on the moving
  operand:
     psum[m=(co*6+r), x] += lhsT_dx[p, m] * window[p, x+dx]
  with block-Toeplitz weights lhsT_dx[k*16+ci, co*6+r] = W[co,ci,k-r,dx]
  (zero outside 0 <= k-r <= 2), precomputed on host from the 9KB weight.
  Contraction K=128, M=96 -> 3 column-streams per 6 output pixels (0.5
  PE-cycles/output-pixel).

  Matmuls run in float16 (1 col/cycle; host pre-casts the shipped windows,
  halving input DMA bytes). Outputs ship as float16 too (rel-err budget is
  2e-2; f16 rounding adds ~1e-4) and are up-cast host-side.

  The default implementation is a raw-bass program (build_raw) with manual
  semaphores: PSUM drains split scalar(0:768)/vector(768:1534) into separate
  per-engine staging tiles, input on the sync HWDGE ring, outputs on the
  scalar ring, weights riding inside group 0's input DMA. Measured 83.5us
  on 8 cores (PE issue-rate floor is ~61us + ~7us framework preamble +
  pipeline fill/drain). The TileContext-based build_program is kept as a
  fallback (CONV_IMPL=tile).
"""

import numpy as np

import concourse.bass as bass
import concourse.mybir as mybir
import concourse.tile as tile
from concourse.bass_utils import run_bass_kernel_spmd

C = 16
H = 1536
W = 1536
HOUT = H - 2
WOUT = W - 2
NCORES = 8
ROWS_PER_CORE = 192  # output rows computed per core
R = 6                # output rows per block
WIN = R + 2          # input rows per window
NBLK = ROWS_PER_CORE // R
XIN_ROWS = ROWS_PER_CORE + 2  # input rows per shard
CHUNKS = [(0, 512), (512, 512), (1024, WOUT - 1024)]
F32 = mybir.dt.float32
F32R = mybir.dt.float32r

_drain_patched = False


def _patch_tile_drain():
    """This container's walrus accepts only ONE sync-wait per lowered
    instruction (CTRL drains, S3_LW weight loads, ...). Tile freely attaches
    several. Split the extras onto single-wait nops placed just before the
    instruction on the same engine (identical blocking semantics)."""
    global _drain_patched
    if _drain_patched:
        return
    _drain_patched = True
    from concourse.tile import ScopedClock

    def _split_multi_waits(ordered):
        for bb_name, insts in ordered.items():
            out = []
            for inst in insts:
                si = getattr(inst, "sync_info", None)
                if (
                    si is not None
                    and si.on_wait is not None
                    and len(si.on_wait) > 1
                    and type(inst).__module__ == "bass_rust"
                ):
                    waits = list(si.on_wait)
                    for i, w in enumerate(waits[:-1]):
                        out.append(
                            mybir.InstNoOp(
                                name=f"{inst.name}ws{i}",
                                engine=inst.engine,
                                bass_nofuse=True,
                                sync_info=mybir.SyncInfo(
                                    on_wait=[w], on_update=[]
                                ),
                            )
                        )
                    inst.sync_info = mybir.SyncInfo(
                        on_wait=[waits[-1]],
                        on_update=list(si.on_update or []),
                    )
                out.append(inst)
            ordered[bb_name] = out
        return ordered

    orig_lower = tile.TileContext._lower_ordered_insts

    def _lower_ordered_insts(self, ordered):
        return orig_lower(self, _split_multi_waits(ordered))

    tile.TileContext._lower_ordered_insts = _lower_ordered_insts

    def _drain_and_barrier(self, tick_clock, wait_clock):
        drain_inst = self.nc.sync.drain()
        wait_clock.add_sem_waits(
            drain_inst.ins, ScopedClock({None: tick_clock.global_clock})
        )
        si = drain_inst.ins.sync_info
        if si is not None and si.on_wait is not None and len(si.on_wait) > 1:
            waits = list(si.on_wait)
            drain_inst.ins.sync_info = mybir.SyncInfo(
                on_wait=[waits[0]], on_update=list(si.on_update or [])
            )
            for w in waits[1:]:
                n = self.nc.sync.nop(nofuse=True, hint="drain_wait_split")
                n.ins.sync_info = mybir.SyncInfo(on_wait=[w], on_update=[])
        self.nc.all_engine_barrier()
        assert self.sems is not None
        popped = self.nc._tile_sem_poison_stack.pop()
        assert popped is self._sem_poison
        self.nc.clear_and_free_semaphores(list(self.sems.allocated().values()))
        self.nc.all_engine_barrier()

    tile.TileContext._drain_and_barrier = _drain_and_barrier


def build_lhsT(weight: np.ndarray) -> np.ndarray:
    """[C_out=16, C_in=16, 3, 3] -> [3, 128, 96] block-Toeplitz stationary
    operands, one per kernel column dx.
    lhsT[dx, ci*8+k, co*6+r] = weight[co, ci, k-r, dx] for 0 <= k-r <= 2."""
    lhsT = np.zeros((3, 128, 96), np.float32)
    ci = np.arange(C)
    co = np.arange(C)
    for dx in range(3):
        for dy in range(3):
            for r in range(R):
                k = r + dy
                lhsT[dx, (ci * WIN + k)[:, None], (co * R + r)[None, :]] = (
                    weight[:, :, dy, dx].T
                )
    return lhsT


def shard_windows(Xs: np.ndarray, dtype=np.float32) -> np.ndarray:
    """Host-side window predup for one core's shard [C, XIN_ROWS, W] ->
    [128, NBLK*W] where out[ci*8+k, b*W+x] = Xs[ci, 6*b+k, x]. dma_start
    carries a large fixed overhead here, so shipping the 33% halo
    duplication in exchange for contiguous multi-MB group DMAs is a clear
    win. For 16-bit matmul dtypes the cast happens here too, halving the
    DMA bytes."""
    rows = (
        R * np.arange(NBLK)[:, None] + np.arange(WIN)[None, :]
    )  # [b, k]
    arr = Xs[:, rows, :].astype(dtype)  # [C, b, k, W]
    arr = arr.transpose(0, 2, 1, 3)  # [C, k, b, W]
    return np.ascontiguousarray(arr.reshape(128, NBLK * W))


def build_program(
    bias_sum: float,
    mm_dtype=F32R,
    nblk=NBLK,
    group=4,
    wbufs=4,
    obufs=4,
    pbufs=2,
    out_dtype=None,
    groups=None,
    repeat=1,
    hw_loop=0,
):
    """One core's program: see module docstring. dma_start carries a large
    fixed overhead in this environment, so inputs arrive as host-preduped
    window groups (one contiguous DMA per `group` blocks) and outputs leave
    as one grouped DMA into a [C, R, nblk, WOUT] device layout the host
    re-transposes (and up-casts when out_dtype is 16-bit — the 2e-2 rel-err
    budget dwarfs f16 rounding, and it halves output DMA bytes).

    Per window, the 3 dx-matmul column-chunks accumulate into ONE 3-bank
    [96, 1536] PSUM tile (chunk boundaries at 512 = bank-aligned), so the
    psum->SBUF drain is a single FD=1534 instruction instead of three;
    drains alternate between the scalar and vector engines so neither sits
    on the critical path. `repeat` (python-unrolled) and `hw_loop`
    (tc.For_i) re-run the whole block sweep, for timing amplification."""
    nc = bass.Bass("TRN2", target_bir_lowering=False, debug=False)
    # variable group sizes: small groups at the start fill the pipeline
    # sooner (first input DMA is on the critical path) and small groups at
    # the end shorten the final drain+DMA tail, while big groups in the
    # middle keep the per-DMA fixed cost amortized.
    if groups is None:
        groups = [2, 2] + [group] * ((nblk - 8) // group) + [2, 2]
    assert sum(groups) == nblk
    gmax = max(groups)
    if out_dtype is None:
        out_dtype = F32
    # 16-bit matmul dtypes are cast host-side: x/wt ship pre-cast, halving
    # input DMA bytes and skipping the on-chip rounding pass. f32r still
    # needs an on-chip DVE rounding producer.
    host_cast = mybir.dt.size(mm_dtype) == 2
    ship_dtype = mm_dtype if host_cast else F32
    x = nc.dram_tensor(
        "x", [128, nblk * W], ship_dtype, kind="ExternalInput"
    ).ap()
    wt = nc.dram_tensor(
        "wt", [3, 128, 96], ship_dtype, kind="ExternalInput"
    ).ap()
    y = nc.dram_tensor(
        "y", [C, R, nblk, WOUT], out_dtype, kind="ExternalOutput"
    ).ap()
    round_on_chip = (not host_cast) and mm_dtype != F32

    with tile.TileContext(nc) as tc:
        with (
            tc.tile_pool(name="wpool", bufs=1) as wpool,
            tc.tile_pool(name="winp", bufs=wbufs) as winp,
            tc.tile_pool(name="opool", bufs=obufs) as opool,
            tc.tile_pool(name="ppool", bufs=pbufs, space="PSUM") as ppool,
        ):
            wts = []
            for dx in range(3):
                if round_on_chip:
                    ws = wpool.tile([128, 96], F32, tag=f"ws{dx}", name=f"ws{dx}")
                    nc.sync.dma_start(ws[:], wt[dx])
                    wtile = wpool.tile(
                        [128, 96], mm_dtype, tag=f"w{dx}", name=f"w{dx}"
                    )
                    nc.vector.tensor_copy(wtile[:], ws[:])
                else:
                    wtile = wpool.tile(
                        [128, 96], mm_dtype, tag=f"w{dx}", name=f"w{dx}"
                    )
                    # gpsimd (SWDGE) queue: the sync queue stays free for the
                    # first input window and the scalar queue for outputs --
                    # tiny weight DMAs starve for ~10us behind multi-MB input
                    # DMAs when they share a ring
                    nc.gpsimd.dma_start(wtile[:], wt[dx])
                wts.append(wtile)

            def sweep(prefix):
                b0 = 0
                for g, nwin in enumerate(groups):
                    uid = f"{prefix}_{g}"
                    # one contiguous DMA loads `nwin` preduped 8-row
                    # windows; partition p = ci*8+k, window w at free cols
                    # [w*W, (w+1)*W). (Splitting this into per-window DMAs
                    # was tried and REGRESSES: the extra per-DMA fixed costs
                    # on the input ring delay later groups by more than the
                    # earlier pipeline fill saves.)
                    win = winp.tile(
                        [128, nwin, W], mm_dtype, tag="win",
                        name=f"win{uid}",
                    )
                    nc.sync.dma_start(
                        win[:],
                        x[:, b0 * W : (b0 + nwin) * W].rearrange(
                            "p (w c) -> p w c", w=nwin
                        ),
                    )

                    # separate low/high column staging tiles so the scalar
                    # and vector drain streams never touch the same tile --
                    # a shared tile couples the two in-order engine queues
                    # through tile-versioning NOPs and stalls the PE at
                    # group boundaries
                    ot_lo = opool.tile(
                        [96, nwin, 768], out_dtype, tag="olo", name=f"olo_{uid}"
                    )
                    ot_hi = opool.tile(
                        [96, nwin, 768], out_dtype, tag="ohi", name=f"ohi_{uid}"
                    )
                    for w in range(nwin):
                        ps = ppool.tile(
                            [96, 3 * 512], F32, tag="ps", name=f"ps_{uid}_{w}"
                        )
                        for dx in range(3):
                            for x0, n in CHUNKS:
                                nc.tensor.matmul(
                                    ps[:, x0 : x0 + n],
                                    wts[dx][:],
                                    win[:, w, x0 + dx : x0 + dx + n],
                                    start=(dx == 0),
                                    stop=(dx == 2),
                                )
                        # both engines drain half each, in parallel: drain
                        # latency ~0.9us << window compute ~1.96us, so the
                        # psum buf is free long before PE needs it again
                        nc.scalar.add(
                            ot_lo[:, w, :], ps[:, 0:768], float(bias_sum)
                        )
                        nc.vector.tensor_scalar_add(
                            ot_hi[:, w, 0 : WOUT - 768],
                            ps[:, 768:WOUT],
                            float(bias_sum),
                        )
                    # two DMAs store `nwin` blocks of 6 output rows into the
                    # [C, R, nblk, WOUT] device layout; (c r) merge matches
                    # the 96-partition source. scalar-engine HWDGE queue
                    # keeps outputs off the input queue.
                    ysl = y[:, :, b0 : b0 + nwin, :]
                    nc.scalar.dma_start(
                        ysl[:, :, :, 0:768].rearrange("c r b x -> (c r) b x"),
                        ot_lo[:96, :, :],
                    )
                    nc.scalar.dma_start(
                        ysl[:, :, :, 768:WOUT].rearrange(
                            "c r b x -> (c r) b x"
                        ),
                        ot_hi[:96, :, 0 : WOUT - 768],
                    )
                    b0 += nwin

            if hw_loop:
                with tc.For_i(
                    0, hw_loop, 1, hint_engines=(mybir.EngineType.PE,)
                ):
                    sweep("L")
            else:
                for rep in range(repeat):
                    sweep(str(rep))
    return nc


GROUPS_RAW = [2, 2, 4, 4, 4, 4, 4, 4, 2, 2]
WARMUP_MMS = 7
F16 = mybir.dt.float16


def build_raw(bias_sum: float, groups=None, wbufs=4, obufs=4):
    """Raw-bass (no TileContext) program: same pipeline as build_program but
    with hand-placed semaphores. Saves ~11us of tile-framework preamble/exit
    machinery and lets the 3 block-Toeplitz weight matrices ride inside
    group 0's input DMA (a separate small weight DMA lands 10-20us late on
    ANY ring while the input ring is blasting multi-MB window transfers,
    stalling the first windows' LDWEIGHTS)."""
    from contextlib import ExitStack

    from concourse.bass import compact_to_ranges

    nc = bass.Bass("TRN2", target_bir_lowering=False, debug=False)
    if groups is None:
        groups = list(GROUPS_RAW)
    assert sum(groups) == NBLK
    gmax = max(groups)
    ng = len(groups)
    b0s = [sum(groups[:g]) for g in range(ng)]
    ends = [b0s[g] + groups[g] for g in range(ng)]

    # x layout: [win0, win1, lhsT(3x96 cols), win2, win3, ...]
    g0w = groups[0]
    x = nc.dram_tensor(
        "x", [128, NBLK * W + 288], F16, kind="ExternalInput"
    ).ap()
    y = nc.dram_tensor(
        "y", [C, R, NBLK, WOUT], F16, kind="ExternalOutput"
    ).ap()

    with ExitStack() as st:
        w0tile = st.enter_context(
            nc.sbuf_tensor("w0tile", [128, g0w * W + 288], F16)
        )
        wtiles = [
            w0tile[:, g0w * W + dx * 96 : g0w * W + (dx + 1) * 96]
            for dx in range(3)
        ]
        wins = [
            st.enter_context(nc.sbuf_tensor(f"win{i}", [128, gmax, W], F16))
            for i in range(wbufs)
        ]
        olos = [
            st.enter_context(nc.sbuf_tensor(f"olo{i}", [96, gmax, 768], F16))
            for i in range(obufs)
        ]
        ohis = [
            st.enter_context(nc.sbuf_tensor(f"ohi{i}", [96, gmax, 768], F16))
            for i in range(obufs)
        ]
        pss = [
            st.enter_context(nc.psum_tensor(f"ps{i}", [96, 1536], F32))
            for i in range(2)
        ]
        # dedicated warmup accumulator: PSUM has_written bits are only
        # cleared when an engine READS the region, so warmup output must
        # never share banks with real windows (a real matmul would
        # accumulate onto the garbage instead of overwriting)
        wups = st.enter_context(nc.psum_tensor("wups", [96, 512], F32))
        sIN = st.enter_context(nc.semaphore(name="sIN"))
        sMM = st.enter_context(nc.semaphore(name="sMM"))
        sACT = st.enter_context(nc.semaphore(name="sACT"))
        sDVE = st.enter_context(nc.semaphore(name="sDVE"))
        sOLO = st.enter_context(nc.semaphore(name="sOLO"))
        sOHI = st.enter_context(nc.semaphore(name="sOHI"))

        # semaphores are NOT zeroed on allocation; reset them and barrier
        # before any engine consumes them (the pattern Bass.__init__ uses
        # for target_bir_lowering=True). The pseudo-barrier alone is not
        # sufficient -- engines can reach their first semaphore wait before
        # gpsimd's clear lands and pass on a STALE value from a previous
        # NEFF execution; the $S-chain all-engine barrier orders every
        # engine after the clear.
        nums = sorted(h.num for h in (sIN, sMM, sACT, sDVE, sOLO, sOHI))
        for rng in compact_to_ranges(nums):
            nc.gpsimd.dma_reset(rng)
            nc.gpsimd.sem_clear(rng)
        nc._nrt_pseudo_barrier()
        nc.all_engine_barrier()

        with nc.Block() as block:

            @block.sync
            def _(eng):
                # group 0: windows + the appended weights in ONE DMA
                eng.dma_start(
                    w0tile[:], x[:, 0 : g0w * W + 288]
                ).then_inc(sIN, 16)
                for g in range(1, ng):
                    if g >= wbufs + 1:
                        # win buffer reuse: all matmuls of the group that
                        # last used this buffer are done
                        eng.wait_ge(sMM, ends[g - wbufs])
                    nwin = groups[g]
                    eng.dma_start(
                        wins[(g - 1) % wbufs][:, 0:nwin, :],
                        x[
                            :, 288 + b0s[g] * W : 288 + ends[g] * W
                        ].rearrange("p (w c) -> p w c", w=nwin),
                    ).then_inc(sIN, 16)
                # final: hold the program open until all outputs landed
                # (last group ships per-window: groups[-1] DMAs per half)
                ndma_out = ng - 1 + groups[-1]
                eng.wait_ge(sOLO, 16 * ndma_out)
                eng.wait_ge(sOHI, 16 * ndma_out)

            @block.tensor
            def _(eng):
                # warmup: dummy matmuls on garbage SBUF while the first
                # input DMA is in flight -- the PE clock ramps 1.2->2.4GHz
                # only after ~3.4us of sustained activity (HAM), so without
                # these the first real windows run at half rate. wins[-1]
                # is not DMA'd into until several groups in, so reading it
                # is race-free (contents irrelevant).
                scratch = wins[wbufs - 1]
                for _ in range(WARMUP_MMS):
                    nc.tensor.matmul(
                        wups[:, 0:512],
                        scratch[:, 0, 0:96],
                        scratch[:, 1, 0:512],
                        start=True,
                        stop=True,
                    )
                i = 0
                for g in range(ng):
                    eng.wait_ge(sIN, 16 * (g + 1))
                    for w in range(groups[g]):
                        if i >= 2:
                            # psum buf reuse: window i-2 fully drained
                            eng.wait_ge(sACT, i - 1)
                            eng.wait_ge(sDVE, i - 1)
                        ps = pss[i % 2]
                        for dx in range(3):
                            for ci, (x0, n) in enumerate(CHUNKS):
                                if g == 0:
                                    rhs = w0tile[
                                        :,
                                        w * W + x0 + dx : w * W + x0 + dx + n,
                                    ]
                                else:
                                    rhs = wins[(g - 1) % wbufs][
                                        :, w, x0 + dx : x0 + dx + n
                                    ]
                                mm = nc.tensor.matmul(
                                    ps[:, x0 : x0 + n],
                                    wtiles[dx],
                                    rhs,
                                    start=(dx == 0),
                                    stop=(dx == 2),
                                )
                                if dx == 2 and ci == 2:
                                    mm.then_inc(sMM, 1)
                        i += 1

            @block.scalar
            def _(eng):
                i = 0
                for g in range(ng):
                    if g >= obufs:
                        # staging tile reuse: group g-obufs's lo DMA done
                        eng.wait_ge(sOLO, 16 * (g - obufs + 1))
                    nwin = groups[g]
                    olo = olos[g % obufs]
                    last = g == ng - 1
                    ysl = y[:, :, b0s[g] : ends[g], :]
                    for w in range(nwin):
                        eng.wait_ge(sMM, i + 1)
                        nc.scalar.add(
                            olo[:, w, :], pss[i % 2][:, 0:768], float(bias_sum)
                        ).then_inc(sACT, 1)
                        i += 1
                        if last:
                            # final group: ship per window so the kernel
                            # tail is one small DMA, not the whole group
                            ywl = ysl[:, :, w : w + 1, :]
                            eng.dma_start(
                                ywl[:, :, :, 0:768].rearrange(
                                    "c r b x -> (c r) b x"
                                ),
                                olo[:, w : w + 1, :],
                            ).then_inc(sOLO, 16)
                            eng.wait_ge(sDVE, i)
                            eng.dma_start(
                                ywl[:, :, :, 768:WOUT].rearrange(
                                    "c r b x -> (c r) b x"
                                ),
                                ohis[g % obufs][
                                    :, w : w + 1, 0 : WOUT - 768
                                ],
                            ).then_inc(sOHI, 16)
                    if last:
                        continue
                    eng.dma_start(
                        ysl[:, :, :, 0:768].rearrange("c r b x -> (c r) b x"),
                        olo[:, 0:nwin, :],
                    ).then_inc(sOLO, 16)
                    # hi-half out-DMA also issued here (only SP/ACT have
                    # HWDGE rings); the sDVE wait is satisfied ~when our own
                    # last drain finishes, so this barely stalls ACT
                    eng.wait_ge(sDVE, ends[g])
                    eng.dma_start(
                        ysl[:, :, :, 768:WOUT].rearrange(
                            "c r b x -> (c r) b x"
                        ),
                        ohis[g % obufs][:, 0:nwin, 0 : WOUT - 768],
                    ).then_inc(sOHI, 16)

            @block.vector
            def _(eng):
                i = 0
                for g in range(ng):
                    if g >= obufs:
                        eng.wait_ge(sOHI, 16 * (g - obufs + 1))
                    ohi = ohis[g % obufs]
                    for w in range(groups[g]):
                        eng.wait_ge(sMM, i + 1)
                        nc.vector.tensor_scalar_add(
                            ohi[:, w, 0 : WOUT - 768],
                            pss[i % 2][:, 768:WOUT],
                            float(bias_sum),
                        ).then_inc(sDVE, 1)
                        i += 1

    return nc


def pack_raw(xs: np.ndarray, lhsT: np.ndarray) -> np.ndarray:
    """[C, XIN_ROWS, W] shard -> the raw program's x layout with the weight
    columns spliced in after group 0's windows."""
    g0w = GROUPS_RAW[0]
    wcols = np.ascontiguousarray(
        lhsT.astype(np.float16).transpose(1, 0, 2).reshape(128, 288)
    )
    xw = shard_windows(xs, np.float16)
    return np.ascontiguousarray(
        np.concatenate([xw[:, : g0w * W], wcols, xw[:, g0w * W :]], axis=1)
    )


def run(X, weight, bias, trace=False, **bkw):
    """Full pipeline; returns (out, BassKernelResults). kernel() wraps it."""
    X = np.ascontiguousarray(np.asarray(X, dtype=np.float32))
    weight = np.asarray(weight, dtype=np.float32)
    bias = np.asarray(bias, dtype=np.float32)

    import os

    lhsT = build_lhsT(weight)
    starts = [min(c * ROWS_PER_CORE, H - XIN_ROWS) for c in range(NCORES)]
    if os.environ.get("CONV_IMPL", "raw") == "raw":
        nc = build_raw(float(bias.sum()))
        in_maps = [
            {"x": pack_raw(X[:, s : s + XIN_ROWS, :], lhsT)} for s in starts
        ]
        last_err = None
        for _ in range(3):
            try:
                res = run_bass_kernel_spmd(
                    nc, in_maps, core_ids=list(range(NCORES)),
                    trace=trace, **bkw,
                )
                break
            except Exception as e:  # noqa: BLE001
                last_err = e
        else:
            raise last_err
        out = np.empty((C, HOUT, WOUT), np.float32)
        for c in range(NCORES):
            yc = np.asarray(res.results[c]["y"], np.float32)
            out[:, starts[c] : starts[c] + ROWS_PER_CORE, :] = (
                yc.transpose(0, 2, 1, 3).reshape(C, ROWS_PER_CORE, WOUT)
            )
        return out, res
    return _run_tile(X, weight, bias, lhsT, starts, trace=trace, **bkw)


def _run_tile(X, weight, bias, lhsT, starts, trace=False, **bkw):
    _patch_tile_drain()
    import os

    mm_dtype = {
        "f32": F32,
        "f32r": F32R,
        "f16": mybir.dt.float16,
        "bf16": mybir.dt.bfloat16,
    }[os.environ.get("CONV_MM_DTYPE", "f16")]
    out_dtype = {
        "f32": F32,
        "f16": mybir.dt.float16,
        "bf16": mybir.dt.bfloat16,
    }[os.environ.get("CONV_OUT_DTYPE", "f16")]
    group = int(os.environ.get("CONV_GROUP", "4"))
    host_cast = mybir.dt.size(mm_dtype) == 2
    ship = mybir.dt.np(mm_dtype) if host_cast else np.float32
    nc = build_program(
        float(bias.sum()), mm_dtype, group=group, out_dtype=out_dtype
    )

    starts = [min(c * ROWS_PER_CORE, H - XIN_ROWS) for c in range(NCORES)]
    in_maps = [
        {
            "x": shard_windows(X[:, s : s + XIN_ROWS, :], ship),
            "wt": lhsT.astype(ship),
        }
        for s in starts
    ]
    # the device occasionally faults transiently (NRT_EXEC_UNIT_UNRECOVERABLE)
    # -- retry a couple of times before giving up
    last_err = None
    for _ in range(3):
        try:
            res = run_bass_kernel_spmd(
                nc, in_maps, core_ids=list(range(NCORES)), trace=trace, **bkw
            )
            break
        except Exception as e:  # noqa: BLE001
            last_err = e
    else:
        raise last_err

    out = np.empty((C, HOUT, WOUT), np.float32)
    for c in range(NCORES):
        yc = np.asarray(res.results[c]["y"], np.float32)  # [C, R, NBLK, WOUT]
        out[:, starts[c] : starts[c] + ROWS_PER_CORE, :] = (
            yc.transpose(0, 2, 1, 3).reshape(C, ROWS_PER_CORE, WOUT)
        )
    return out, res


def kernel(X: np.ndarray, weight: np.ndarray, bias: np.ndarray) -> np.ndarray:
    out, _ = run(X, weight, bias)
    return out

